# revision 13
# baseline (speedup 1.0000x reference)
"""CP-factorized embedding lookup on 8 TRN2 NeuronCores (v3).

Reference computes full[a,b,c,d,e,f] = sum_r U0[a,r]*...*U5[f,r], reshapes to a
(50000, 512) table, and gathers rows by x. We never materialize the table:

  out[n, e] = sum_r (U0[a_n,r]*U1[b_n,r]*U2[c_n,r]) * (U3[d,r]*U4[e2,r]*U5[f,r])
            = sum_r V[n, r] * W[e, r]

with v = 1000a + 25b + c and e = 64d + 8e2 + f.

Per core (1024 indices):
  1. x lands once as [8, 128] (4 KB). Digits via exact float-floor tricks,
     split across Scalar-ACT (q=floor(v/25), 25q) and DVE tensor_scalar
     (a=floor(v/1000), 40a, m3=1000*min(v,1)); three DVE tensor_tensor ops
     write h0=a-m3, h1=q-40a, h2=v-25q into a zeroed [96, 128] fp16 tile at
     partition bases 0/32/64 (engine APs must start at multiples of 32).
     Padding folds in: v==0 gives h0=0 which misses every block-0 target
     (iota0 = p-1000), so V0=0 and the output row is zero.
  2. Eight PE matmuls with 0/1 selector weights B96_j broadcast the h rows
     across the 115 stacked vocab-factor partitions; one is_equal
     tensor_scalar per 512-half against the per-partition local index gives
     the fp16 one-hot.
  3. One PE matmul per half gathers all three factor rows at once (block-diag
     [U0;U1;U2] stationary) -> psum[96, 512]; V = product of the three 32-row
     blocks (one scalar copy + two DVE multiplies reading psum).
  4. out chunk: matmul(lhsT=V[:, 128-slice], rhs=W[32, 512]) -> psum, two
     chunks per [128, 1024] psum pair; psum->SBUF fp16 copies split between
     Scalar and Vector; one DMA per 256 output rows.

Schedule notes (all from the HW profile): the B96 selector table (176 KB of
the aux input) is DMA'd as two column-halves on separate rings in parallel
with the small table and x (a single [115, 2 KB] HWDGE DMA only engages ~5
SDMA engines at ~90 GB/s); 28 junk matmuls on memset tiles run first on the
otherwise-idle PE to trip the HAM clock gate from 1.2 to 2.4 GHz before the
real matmuls; W-build and small casts run on the otherwise-idle GpSimd; the
four output DMAs alternate sync/gpsimd queues. Output is fp16 (1 MB instead
of 2 MB of HBM writes; ~2^-11 rounding vs the 2e-2 tolerance), cast to f32
on host. Host-side work is dtype casting and zero-padded packing only.
"""

import numpy as np

import concourse.bass as bass
import concourse.mybir as mybir
import concourse.tile as tile
from concourse import bacc
from concourse.bass_utils import run_bass_kernel_spmd

F32 = mybir.dt.float32
F16 = mybir.dt.float16
I32 = mybir.dt.int32
I16 = mybir.dt.int16
ALU = mybir.AluOpType
ACT = mybir.ActivationFunctionType

N_CORES = 8
PER_CORE = 1024           # indices per core (8192 / 8)
HALF = 512
EMB = 512
RANK = 32
KV = 115                  # 50 + 40 + 25 stacked vocab-factor rows
MV = 96                   # 3 * RANK stacked gather outputs
NJ = 8                    # index chunks of 128 (x laid out [8, 128])

R25 = float(np.float32(1.0 / 25.0))
R1000 = float(np.float32(1.0 / 1000.0))

# auxh (fp16) layout: [115, AUXH_W]; small tables first so one small DMA
# covers them, the 8 B96_j selectors after (split into two DMAs).
UBLK_OFF = 0     # rows 0:115, cols 0:96      block-diag [U0;U1;U2]
U345_OFF = 96    # rows 0:24,  cols 96:128    stacked U3;U4;U5
ID24_OFF = 128   # rows 0:24,  cols 128:152   identity 24
IOTA_OFF = 152   # rows 0:115, col 152        per-partition compare target
SMALL_W = 154    # pad to even
B96_OFF = 154    # rows 0:96,  cols 154:1074  8x B96_j [96, 115]
AUXH_W = 1074


def _auxh_table(us: list[np.ndarray]) -> np.ndarray:
    aux = np.zeros((KV, AUXH_W), np.float16)
    aux[0:50, UBLK_OFF:UBLK_OFF + 32] = us[0].astype(np.float16)
    aux[50:90, UBLK_OFF + 32:UBLK_OFF + 64] = us[1].astype(np.float16)
    aux[90:115, UBLK_OFF + 64:UBLK_OFF + 96] = us[2].astype(np.float16)
    aux[0:8, U345_OFF:U345_OFF + 32] = us[3].astype(np.float16)
    aux[8:16, U345_OFF:U345_OFF + 32] = us[4].astype(np.float16)
    aux[16:24, U345_OFF:U345_OFF + 32] = us[5].astype(np.float16)
    aux[0:24, ID24_OFF:ID24_OFF + 24] = np.eye(24, dtype=np.float16)
    # block-0 compares h0 = a - m3 = a - 1000 (v>=1), so target is p - 1000;
    # v==0 gives h0 = 0 which misses all of [-1000, -951] -> zero row
    iota = np.concatenate([np.arange(50) - 1000, np.arange(40), np.arange(25)])
    aux[:, IOTA_OFF] = iota.astype(np.float16)
    # B96_j[k, p] = 1 iff k == 32*block(p) + j (h rows at bases 0/32/64)
    blk = np.zeros(KV, np.int64)
    blk[50:90] = 1
    blk[90:115] = 2
    for j in range(NJ):
        m = np.zeros((96, KV), np.float16)
        m[32 * blk + j, np.arange(KV)] = 1.0
        aux[0:96, B96_OFF + KV * j:B96_OFF + KV * (j + 1)] = m
    return aux


def build():
    nc = bacc.Bacc("TRN2", target_bir_lowering=False, debug=False)

    x = nc.dram_tensor("x", [PER_CORE], I32, kind="ExternalInput")
    auxh_d = nc.dram_tensor("auxh", [KV, AUXH_W], F16, kind="ExternalInput")
    out = nc.dram_tensor("out", [PER_CORE, EMB], F16, kind="ExternalOutput")
    outv = out[:].rearrange("(j p) e -> p j e", p=128)  # partition p, row 128j+p

    B96_MID = B96_OFF + KV * 4

    with tile.TileContext(nc) as tc:
        with (
            tc.tile_pool(name="const", bufs=1) as cpool,
            tc.tile_pool(name="work", bufs=2) as wpool,
            tc.tile_pool(name="ow", bufs=8) as opool,
            tc.tile_pool(name="pbc", bufs=2, space="PSUM") as bcpool,
            tc.tile_pool(name="pv", bufs=2, space="PSUM") as pvpool,
            tc.tile_pool(name="po", bufs=4, space="PSUM") as popool,
        ):
            auxh = cpool.tile([KV, AUXH_W], F16)

            # ---- input DMAs: x + two B96 halves + small tables, on three
            # queues so no single ~90 GB/s HWDGE stream is the bottleneck.
            xt = cpool.tile([NJ, 128], I32)
            nc.sync.dma_start(out=xt[:], in_=x[:].rearrange("(j n) -> j n", j=NJ))
            B96_T = B96_OFF + KV * 6
            nc.scalar.dma_start(out=auxh[0:96, B96_OFF:B96_MID],
                                in_=auxh_d[0:96, B96_OFF:B96_MID])
            nc.sync.dma_start(out=auxh[0:96, B96_T:AUXH_W],
                              in_=auxh_d[0:96, B96_T:AUXH_W])

            # ---- GpSimd stream: junk tiles for PE warm-up, h3T zero-fill,
            # second B96 half, iota cast, W-build products.
            h3T = cpool.tile([MV, 128], F16)
            nc.gpsimd.memset(h3T[:], 0.0)
            nc.gpsimd.dma_start(out=auxh[:, 0:SMALL_W],
                                in_=auxh_d[:, 0:SMALL_W])
            nc.gpsimd.dma_start(out=auxh[0:96, B96_MID:B96_T],
                                in_=auxh_d[0:96, B96_MID:B96_T])

            ublk = auxh[:, UBLK_OFF:UBLK_OFF + MV]
            u345 = auxh[0:24, U345_OFF:U345_OFF + 32]
            id24 = auxh[0:24, ID24_OFF:ID24_OFF + 24]
            iota16 = auxh[:, IOTA_OFF:IOTA_OFF + 1]

            iota = cpool.tile([KV, 1], F32)
            nc.gpsimd.tensor_copy(out=iota[:], in_=iota16)

            # ---- digit decomposition, Scalar and Vector in parallel:
            # S: q = floor(v/25), q25 = 25q;  V: a = floor(v/1000), 40a, m3
            q = cpool.tile([NJ, 128], I16)
            nc.scalar.activation(q[:], xt[:], ACT.Copy,
                                 bias=float(np.float32(-12.0 * R25)), scale=R25)
            q25 = cpool.tile([NJ, 128], I32)
            nc.scalar.activation(q25[:], q[:], ACT.Copy, bias=0.0, scale=25.0)
            a = cpool.tile([NJ, 128], I16)
            nc.vector.tensor_scalar(out=a[:], in0=xt[:], scalar1=-499.5,
                                    scalar2=R1000, op0=ALU.add, op1=ALU.mult)
            a40 = cpool.tile([NJ, 128], I16)
            nc.vector.tensor_scalar(out=a40[:], in0=a[:], scalar1=40.0,
                                    scalar2=None, op0=ALU.mult)
            m3 = cpool.tile([NJ, 128], I16)
            nc.vector.tensor_scalar(out=m3[:], in0=xt[:], scalar1=1.0,
                                    scalar2=1000.0, op0=ALU.min, op1=ALU.mult)

            # h3T rows 0:8 = h0 = a - m3, 32:40 = h1 = q - 40a, 64:72 = h2
            nc.vector.tensor_tensor(out=h3T[0:8, :], in0=a[:], in1=m3[:],
                                    op=ALU.subtract)
            nc.vector.tensor_tensor(out=h3T[32:40, :], in0=q[:], in1=a40[:],
                                    op=ALU.subtract)
            nc.vector.tensor_tensor(out=h3T[64:72, :], in0=xt[:], in1=q25[:],
                                    op=ALU.subtract)

            # ---- W[r, e] = U3[d,r]*U4[e2,r]*U5[f,r],  e = 64d + 8e2 + f
            u345t_ps = pvpool.tile([RANK, 24], F16, tag="pv")
            nc.tensor.transpose(u345t_ps[:], u345, id24)
            u345t = cpool.tile([RANK, 24], F16)
            nc.vector.tensor_copy(out=u345t[:], in_=u345t_ps[:])
            t45 = cpool.tile([RANK, 64], F16)
            nc.gpsimd.tensor_tensor(
                out=t45[:].rearrange("r (e f) -> r e f", e=8),
                in0=u345t[:, 8:16].unsqueeze(2).broadcast_to([RANK, 8, 8]),
                in1=u345t[:, 16:24].unsqueeze(1).broadcast_to([RANK, 8, 8]),
                op=ALU.mult,
            )
            wt = cpool.tile([RANK, EMB], F16)
            nc.gpsimd.tensor_tensor(
                out=wt[:].rearrange("r (d ef) -> r d ef", d=8),
                in0=u345t[:, 0:8].unsqueeze(2).broadcast_to([RANK, 8, 64]),
                in1=t45[:].unsqueeze(1).broadcast_to([RANK, 8, 64]),
                op=ALU.mult,
            )

            # ---- broadcast h across factor rows + one-hot compare,
            # then per half: gather, 3-way product, output matmuls, store.
            onehot = cpool.tile([KV, PER_CORE], F16)
            pbc, pv, s0, v01, vth = [], [], [], [], []
            for h in range(2):
                p = bcpool.tile([KV, HALF], F32, name=f"pbc_{h}", tag="pbc")
                pbc.append(p)
                for jl in range(4):
                    j = 4 * h + jl
                    nc.tensor.matmul(
                        p[:, jl * 128:(jl + 1) * 128],
                        lhsT=auxh[0:96, B96_OFF + KV * j:B96_OFF + KV * (j + 1)],
                        rhs=h3T[:],
                        start=True, stop=True,
                    )
                nc.vector.tensor_scalar(
                    out=onehot[:, h * HALF:(h + 1) * HALF], in0=p[:],
                    scalar1=iota[:, 0:1], scalar2=None, op0=ALU.is_equal,
                )
                t = pvpool.tile([MV, HALF], F32, name=f"pv_{h}", tag="pv")
                pv.append(t)
                nc.tensor.matmul(
                    t[:], lhsT=ublk,
                    rhs=onehot[:, h * HALF:(h + 1) * HALF],
                    start=True, stop=True,
                )
                t = wpool.tile([RANK, HALF], F16, name=f"s0_{h}", tag="s0")
                s0.append(t)
                nc.scalar.copy(out=t[:], in_=pv[h][0:32, :])
            for h in range(2):
                t = wpool.tile([RANK, HALF], F16, name=f"v01_{h}", tag="v01")
                v01.append(t)
                nc.vector.tensor_tensor(out=t[:], in0=s0[h][:],
                                        in1=pv[h][32:64, :], op=ALU.mult)
                t = wpool.tile([RANK, HALF], F16, name=f"vth_{h}", tag="vth")
                vth.append(t)
                nc.vector.tensor_tensor(out=t[:], in0=v01[h][:],
                                        in1=pv[h][64:96, :], op=ALU.mult)

            for k in range(8):
                j = k
                h = k // 4
                po = popool.tile([128, EMB], F32, name=f"po_{k}", tag="po")
                nc.tensor.matmul(
                    po[:], lhsT=vth[h][:, (k % 4) * 128:(k % 4 + 1) * 128],
                    rhs=wt[:], start=True, stop=True,
                )
                osb = opool.tile([128, EMB], F16, name=f"osb_{k}", tag="osb")
                # evacuate psum alternating Scalar/Vector; DMA each 128-row
                # slab as soon as its copy lands, alternating queues
                if k % 2 == 0:
                    nc.scalar.copy(out=osb[:], in_=po[:])
                    nc.sync.dma_start(out=outv[:, j:j + 1, :],
                                      in_=osb[:].unsqueeze(1))
                else:
                    nc.vector.tensor_copy(out=osb[:], in_=po[:])
                    nc.gpsimd.dma_start(out=outv[:, j:j + 1, :],
                                        in_=osb[:].unsqueeze(1))

    nc.compile()
    return nc


_CACHE: dict = {}


def _get_nc():
    if "nc" not in _CACHE:
        _CACHE["nc"] = build()
    return _CACHE["nc"]


def run(inputs, **spmd_kwargs):
    nc = _get_nc()
    x = np.ascontiguousarray(inputs["x"].reshape(-1), dtype=np.int32)
    us = [
        np.ascontiguousarray(inputs[f"U{j}"], dtype=np.float32) for j in range(6)
    ]
    auxh = _auxh_table(us)
    in_maps = []
    for i in range(N_CORES):
        in_maps.append({"x": x[i * PER_CORE:(i + 1) * PER_CORE], "auxh": auxh})
    res = run_bass_kernel_spmd(
        nc, in_maps, core_ids=list(range(N_CORES)), **spmd_kwargs
    )
    shards = [np.asarray(res.results[i]["out"]) for i in range(N_CORES)]
    full = np.concatenate(shards, axis=0).reshape(4, 2048, EMB)
    return full.astype(np.float32), res


def kernel(**inputs) -> np.ndarray:
    return run(inputs)[0]


# revision 14
# speedup vs baseline: 1.0625x; 1.0625x over previous
"""CP-factorized embedding lookup on 8 TRN2 NeuronCores (v3).

Reference computes full[a,b,c,d,e,f] = sum_r U0[a,r]*...*U5[f,r], reshapes to a
(50000, 512) table, and gathers rows by x. We never materialize the table:

  out[n, e] = sum_r (U0[a_n,r]*U1[b_n,r]*U2[c_n,r]) * (U3[d,r]*U4[e2,r]*U5[f,r])
            = sum_r V[n, r] * W[e, r]

with v = 1000a + 25b + c and e = 64d + 8e2 + f.

Per core (1024 indices):
  1. x lands once as [8, 128] (4 KB). Digits via exact float-floor tricks,
     split across Scalar-ACT (q=floor(v/25), 25q) and DVE tensor_scalar
     (a=floor(v/1000), 40a, m3=1000*min(v,1)); three DVE tensor_tensor ops
     write h0=a-m3, h1=q-40a, h2=v-25q into a zeroed [96, 128] fp16 tile at
     partition bases 0/32/64 (engine APs must start at multiples of 32).
     Padding folds in: v==0 gives h0=0 which misses every block-0 target
     (iota0 = p-1000), so V0=0 and the output row is zero.
  2. Eight PE matmuls with 0/1 selector weights B96_j broadcast the h rows
     across the 115 stacked vocab-factor partitions; one is_equal
     tensor_scalar per 512-half against the per-partition local index gives
     the fp16 one-hot.
  3. One PE matmul per half gathers all three factor rows at once (block-diag
     [U0;U1;U2] stationary) -> psum[96, 512]; V = product of the three 32-row
     blocks (one scalar copy + two DVE multiplies reading psum).
  4. out chunk: matmul(lhsT=V[:, 128-slice], rhs=W[32, 512]) -> psum, two
     chunks per [128, 1024] psum pair; psum->SBUF fp16 copies split between
     Scalar and Vector; one DMA per 256 output rows.

Schedule notes (all from the HW profile): the B96 selector table (176 KB of
the aux input) is DMA'd as two column-halves on separate rings in parallel
with the small table and x (a single [115, 2 KB] HWDGE DMA only engages ~5
SDMA engines at ~90 GB/s); 28 junk matmuls on memset tiles run first on the
otherwise-idle PE to trip the HAM clock gate from 1.2 to 2.4 GHz before the
real matmuls; W-build and small casts run on the otherwise-idle GpSimd; the
four output DMAs alternate sync/gpsimd queues. Output is fp16 (1 MB instead
of 2 MB of HBM writes; ~2^-11 rounding vs the 2e-2 tolerance), cast to f32
on host. Host-side work is dtype casting and zero-padded packing only.
"""

import numpy as np

import concourse.bass as bass
import concourse.mybir as mybir
import concourse.tile as tile
from concourse import bacc
from concourse.bass_utils import run_bass_kernel_spmd

F32 = mybir.dt.float32
F16 = mybir.dt.float16
I32 = mybir.dt.int32
I16 = mybir.dt.int16
ALU = mybir.AluOpType
ACT = mybir.ActivationFunctionType

N_CORES = 8
PER_CORE = 1024           # indices per core (8192 / 8)
HALF = 512
EMB = 512
RANK = 32
KV = 115                  # 50 + 40 + 25 stacked vocab-factor rows
MV = 96                   # 3 * RANK stacked gather outputs
NJ = 8                    # index chunks of 128 (x laid out [8, 128])

R25 = float(np.float32(1.0 / 25.0))
R1000 = float(np.float32(1.0 / 1000.0))

# auxh (fp16) layout: [115, AUXH_W]; small tables first so one small DMA
# covers them, the 8 B96_j selectors after (split into two DMAs).
UBLK_OFF = 0     # rows 0:115, cols 0:96      block-diag [U0;U1;U2]
U345_OFF = 96    # rows 0:32,  cols 96:120    stacked U3^T;U4^T;U5^T  [32, 24]
IOTA_OFF = 120   # rows 0:115, col 120        per-partition compare target
SMALL_W = 122    # pad to even
B96_OFF = 122    # rows 0:96,  cols 122:1042  8x B96_j [96, 115]
AUXH_W = 1042


def _auxh_table(us: list[np.ndarray]) -> np.ndarray:
    aux = np.zeros((KV, AUXH_W), np.float16)
    aux[0:50, UBLK_OFF:UBLK_OFF + 32] = us[0].astype(np.float16)
    aux[50:90, UBLK_OFF + 32:UBLK_OFF + 64] = us[1].astype(np.float16)
    aux[90:115, UBLK_OFF + 64:UBLK_OFF + 96] = us[2].astype(np.float16)
    aux[0:32, U345_OFF:U345_OFF + 8] = us[3].T.astype(np.float16)
    aux[0:32, U345_OFF + 8:U345_OFF + 16] = us[4].T.astype(np.float16)
    aux[0:32, U345_OFF + 16:U345_OFF + 24] = us[5].T.astype(np.float16)
    # block-0 compares h0 = a - m3 = a - 1000 (v>=1), so target is p - 1000;
    # v==0 gives h0 = 0 which misses all of [-1000, -951] -> zero row
    iota = np.concatenate([np.arange(50) - 1000, np.arange(40), np.arange(25)])
    aux[:, IOTA_OFF] = iota.astype(np.float16)
    # B96_j[k, p] = 1 iff k == 32*block(p) + j (h rows at bases 0/32/64)
    blk = np.zeros(KV, np.int64)
    blk[50:90] = 1
    blk[90:115] = 2
    for j in range(NJ):
        m = np.zeros((96, KV), np.float16)
        m[32 * blk + j, np.arange(KV)] = 1.0
        aux[0:96, B96_OFF + KV * j:B96_OFF + KV * (j + 1)] = m
    return aux


def build():
    nc = bacc.Bacc("TRN2", target_bir_lowering=False, debug=False)

    x = nc.dram_tensor("x", [PER_CORE], I32, kind="ExternalInput")
    auxh_d = nc.dram_tensor("auxh", [KV, AUXH_W], F16, kind="ExternalInput")
    out = nc.dram_tensor("out", [PER_CORE, EMB], F16, kind="ExternalOutput")
    outv = out[:].rearrange("(j p) e -> p j e", p=128)  # partition p, row 128j+p

    B96_MID = B96_OFF + KV * 4

    with tile.TileContext(nc) as tc:
        with (
            tc.tile_pool(name="const", bufs=1) as cpool,
            tc.tile_pool(name="work", bufs=2) as wpool,
            tc.tile_pool(name="ow", bufs=8) as opool,
            tc.tile_pool(name="pbc", bufs=6, space="PSUM") as bcpool,
            tc.tile_pool(name="pv", bufs=2, space="PSUM") as pvpool,
        ):
            auxh = cpool.tile([KV, AUXH_W], F16)

            # ---- input DMAs: x + two B96 halves + small tables, on three
            # queues so no single ~90 GB/s HWDGE stream is the bottleneck.
            xt = cpool.tile([NJ, 128], I32)
            nc.sync.dma_start(out=xt[:], in_=x[:].rearrange("(j n) -> j n", j=NJ))
            B96_T = B96_OFF + KV * 6
            nc.scalar.dma_start(out=auxh[0:96, B96_OFF:B96_MID],
                                in_=auxh_d[0:96, B96_OFF:B96_MID])
            nc.sync.dma_start(out=auxh[0:96, B96_T:AUXH_W],
                              in_=auxh_d[0:96, B96_T:AUXH_W])

            # ---- GpSimd stream: junk tiles for PE warm-up, h3T zero-fill,
            # second B96 half, iota cast, W-build products.
            h3T = cpool.tile([MV, 128], F16)
            nc.gpsimd.memset(h3T[:], 0.0)
            nc.gpsimd.dma_start(out=auxh[:, 0:SMALL_W],
                                in_=auxh_d[:, 0:SMALL_W])
            nc.gpsimd.dma_start(out=auxh[0:96, B96_MID:B96_T],
                                in_=auxh_d[0:96, B96_MID:B96_T])

            ublk = auxh[:, UBLK_OFF:UBLK_OFF + MV]
            u345t = auxh[0:32, U345_OFF:U345_OFF + 24]
            iota16 = auxh[:, IOTA_OFF:IOTA_OFF + 1]

            iota = cpool.tile([KV, 1], F32)
            nc.gpsimd.tensor_copy(out=iota[:], in_=iota16)

            # ---- digit decomposition, Scalar and Vector in parallel:
            # S: q = floor(v/25), q25 = 25q;  V: a = floor(v/1000), 40a, m3
            q = cpool.tile([NJ, 128], I16)
            nc.scalar.activation(q[:], xt[:], ACT.Copy,
                                 bias=float(np.float32(-12.0 * R25)), scale=R25)
            q25 = cpool.tile([NJ, 128], I32)
            nc.scalar.activation(q25[:], q[:], ACT.Copy, bias=0.0, scale=25.0)
            a = cpool.tile([NJ, 128], I16)
            nc.vector.tensor_scalar(out=a[:], in0=xt[:], scalar1=-499.5,
                                    scalar2=R1000, op0=ALU.add, op1=ALU.mult)
            a40 = cpool.tile([NJ, 128], I16)
            nc.vector.tensor_scalar(out=a40[:], in0=a[:], scalar1=40.0,
                                    scalar2=None, op0=ALU.mult)
            m3 = cpool.tile([NJ, 128], I16)
            nc.vector.tensor_scalar(out=m3[:], in0=xt[:], scalar1=1.0,
                                    scalar2=1000.0, op0=ALU.min, op1=ALU.mult)

            # h3T rows 0:8 = h0 = a - m3, 32:40 = h1 = q - 40a, 64:72 = h2
            nc.vector.tensor_tensor(out=h3T[0:8, :], in0=a[:], in1=m3[:],
                                    op=ALU.subtract)
            nc.vector.tensor_tensor(out=h3T[32:40, :], in0=q[:], in1=a40[:],
                                    op=ALU.subtract)
            nc.vector.tensor_tensor(out=h3T[64:72, :], in0=xt[:], in1=q25[:],
                                    op=ALU.subtract)

            # ---- W[r, e] = U3[d,r]*U4[e2,r]*U5[f,r],  e = 64d + 8e2 + f
            t45 = cpool.tile([RANK, 64], F16)
            nc.gpsimd.tensor_tensor(
                out=t45[:].rearrange("r (e f) -> r e f", e=8),
                in0=u345t[:, 8:16].unsqueeze(2).broadcast_to([RANK, 8, 8]),
                in1=u345t[:, 16:24].unsqueeze(1).broadcast_to([RANK, 8, 8]),
                op=ALU.mult,
            )
            wt = cpool.tile([RANK, EMB], F16)
            nc.gpsimd.tensor_tensor(
                out=wt[:].rearrange("r (d ef) -> r d ef", d=8),
                in0=u345t[:, 0:8].unsqueeze(2).broadcast_to([RANK, 8, 64]),
                in1=t45[:].unsqueeze(1).broadcast_to([RANK, 8, 64]),
                op=ALU.mult,
            )

            # ---- broadcast h across factor rows + one-hot compare,
            # then per half: gather, 3-way product, output matmuls, store.
            onehot = cpool.tile([KV, PER_CORE], F16)
            pbc, pv, s0, v01, vth = [], [], [], [], []
            for h in range(2):
                p = bcpool.tile([KV, HALF], F32, name=f"pbc_{h}", tag="pbc")
                pbc.append(p)
                for jl in range(4):
                    j = 4 * h + jl
                    nc.tensor.matmul(
                        p[:, jl * 128:(jl + 1) * 128],
                        lhsT=auxh[0:96, B96_OFF + KV * j:B96_OFF + KV * (j + 1)],
                        rhs=h3T[:],
                        start=True, stop=True,
                    )
                nc.vector.tensor_scalar(
                    out=onehot[:, h * HALF:(h + 1) * HALF], in0=p[:],
                    scalar1=iota[:, 0:1], scalar2=None, op0=ALU.is_equal,
                )
                t = pvpool.tile([MV, HALF], F32, name=f"pv_{h}", tag="pv")
                pv.append(t)
                nc.tensor.matmul(
                    t[:], lhsT=ublk,
                    rhs=onehot[:, h * HALF:(h + 1) * HALF],
                    start=True, stop=True,
                )
                t = wpool.tile([RANK, HALF], F16, name=f"s0_{h}", tag="s0")
                s0.append(t)
                nc.scalar.copy(out=t[:, 0:256], in_=pv[h][0:32, 0:256])
                nc.scalar.copy(out=t[:, 256:512], in_=pv[h][0:32, 256:512])
            for h in range(2):
                tv = wpool.tile([RANK, HALF], F16, name=f"v01_{h}", tag="v01")
                v01.append(tv)
                tw = wpool.tile([RANK, HALF], F16, name=f"vth_{h}", tag="vth")
                vth.append(tw)
                for c in (slice(0, 256), slice(256, 512)):
                    nc.vector.tensor_tensor(out=tv[:, c], in0=s0[h][:, c],
                                            in1=pv[h][32:64, c], op=ALU.mult)
                    nc.vector.tensor_tensor(out=tw[:, c], in0=tv[:, c],
                                            in1=pv[h][64:96, c], op=ALU.mult)

            for k in range(8):
                j = k
                h = k // 4
                po = bcpool.tile([128, EMB], F32, name=f"po_{k}", tag="pbc")
                nc.tensor.matmul(
                    po[:], lhsT=vth[h][:, (k % 4) * 128:(k % 4 + 1) * 128],
                    rhs=wt[:], start=True, stop=True,
                )
                osb = opool.tile([128, EMB], F16, name=f"osb_{k}", tag="osb")
                # evacuate psum alternating Scalar/Vector; DMA each 128-row
                # slab as soon as its copy lands, alternating queues
                if k in (0, 1, 2, 4, 6):
                    nc.scalar.copy(out=osb[:], in_=po[:])
                else:
                    nc.vector.tensor_copy(out=osb[:], in_=po[:])
                eng = nc.sync if k % 2 == 0 else nc.gpsimd
                eng.dma_start(out=outv[:, j:j + 1, :],
                              in_=osb[:].unsqueeze(1))

    nc.compile()
    return nc


_CACHE: dict = {}


def _get_nc():
    if "nc" not in _CACHE:
        _CACHE["nc"] = build()
    return _CACHE["nc"]


def run(inputs, **spmd_kwargs):
    nc = _get_nc()
    x = np.ascontiguousarray(inputs["x"].reshape(-1), dtype=np.int32)
    us = [
        np.ascontiguousarray(inputs[f"U{j}"], dtype=np.float32) for j in range(6)
    ]
    auxh = _auxh_table(us)
    in_maps = []
    for i in range(N_CORES):
        in_maps.append({"x": x[i * PER_CORE:(i + 1) * PER_CORE], "auxh": auxh})
    res = run_bass_kernel_spmd(
        nc, in_maps, core_ids=list(range(N_CORES)), **spmd_kwargs
    )
    shards = [np.asarray(res.results[i]["out"]) for i in range(N_CORES)]
    full = np.concatenate(shards, axis=0).reshape(4, 2048, EMB)
    return full.astype(np.float32), res


def kernel(**inputs) -> np.ndarray:
    return run(inputs)[0]
